# revision 24
# baseline (speedup 1.0000x reference)
"""Trainium2 Bass kernel for nn_MoEsparseRoutingForClassification.

Reference computation (B=64, S=128, H=1024, E=8, L=2):
    x = X[:, 0, :]                                   # CLS token [B,H]
    y[b,o]   = sum_e g[b,e] * (x[b] . dense_w[e,o,:]) + (g @ dense_b)[b,o]
    t        = tanh(y)
    out[b,l] = sum_e g[b,e] * (t[b] . out_w[e,l,:])  + (g @ out_b)[b,l]

Distribution: the H output dim of the dense layer is sharded 8 ways
(OC=128 per core).  Core c computes y[:, c*OC:(c+1)*OC], applies tanh,
and contracts against out_w[:, :, c_slice] for a partial [B,L] logit;
partials sum on the host.  No cross-core collective.

All streamed tensors are bf16 (host cast; tolerance 2e-2, bf16 lands
~5e-3): HBM traffic halves to ~2.2 MiB/core and every matmul is a
single-pass bf16 op.  The CLS block (xt) and w1 ride ONE packed DRAM
tensor, streamed on the sync ring in 4 chunks the PE chases.

Scheduling notes (from NTFF traces; measured 22.9us vs 31.8us for the
fp32 predecessor, of which ~2.5us head and ~7us NEFF-teardown cascade
are fixed framework costs inside the measured exec window):
  - the two HW-DGE rings (sync/SP + scalar/Activation) share one
    descriptor-processing pool (~420 GB/s aggregate, and the scalar
    ring only gets service after sync's queued work), so weights all
    go on the sync ring; more than ~4 chunk boundaries costs ~25%
    stream rate;
  - every DMA's completion semaphore reaches its 16-increment target
    ~1.1us AFTER its last byte (fixed descriptor-completion pacing),
    so consumers see chunk-done = bytes + ~1.1us; the final k7-only
    chunk keeps that exposed lag to one k-pair;
  - tiny-descriptor transfers (ep/gc) stall a weight queue mid-stream
    and complete very late on the handicapped scalar ring - gc+ep ride
    the scalar ring, whose first ~2us are eaten by the tanh
    ACT_TABLE_LOAD's own DMA anyway;
  - gpsimd TENSOR_SCALAR is ~1.15us/op (5x vector) and gpsimd cannot
    touch PSUM - gb builds on the DVE, where it hides under the stream;
  - the 1 KiB output DMA is emitted AFTER the TileContext closes
    (raw sbuf staging tensor + manual .then_inc sem): the tile
    teardown would otherwise serially wait ~3us for its descriptor
    completions; untracked, they hide under the teardown's per-engine
    semaphore-reset cascade, and the tile end barrier still orders the
    dispatch after the final DVE op.  The NEFF's last instruction
    retires ~6us after the bytes land, so the output is quiescent well
    before the runtime reads it.

On-core layout: OC=128 splits into two 64-wide halves mapped to PSUM
partition halves (rows 64h+b); the two matmul streams per k-tile write
disjoint PE column groups which the PE runs concurrently (426ns per
N=512 bf16 pair).

Tail: bf16 pairwise tree for the gate mix; out_b folded into stage 2
via a constant-1 column appended to tanh(y) and sel_ob as column HC of
the sel_ow SBUF copy (copy also lets stage 2 read SBUF, not PSUM).
"""

import sys

import numpy as np

for _p in ("/opt/trn_rl_repo",):
    if _p not in sys.path:
        sys.path.insert(0, _p)

# If the environment sets BASS_TRACE but lacks antenv.axon_hooks (this agent
# image does), run_bass_kernel_spmd would crash on import; pre-seed a no-op
# module so tracing degrades gracefully instead.
try:  # pragma: no cover
    import antenv.axon_hooks  # noqa: F401
except Exception:  # pragma: no cover
    import types as _types

    _m = _types.ModuleType("antenv.axon_hooks")
    _m._hook = None
    _m.set_axon_ntff_profile_hook = lambda h: setattr(_m, "_hook", h)
    _m.get_axon_ntff_profile_hook = lambda: _m._hook
    sys.modules["antenv.axon_hooks"] = _m

from ml_dtypes import bfloat16  # noqa: E402

B, S, H = 64, 128, 1024
E, L = 8, 2
NCORES = 8
OC = H // NCORES          # dense-output slice per core (128)
HC = OC // 2              # half-slice mapped to a PSUM partition half (64)
KT = H // 128             # contraction tiles
P = 128

XTC = KT * B              # xt cols in the packed stream (512)
W1C = KT * 2 * E * HC     # w1 cols (8192)
XWC = XTC + W1C           # packed stream width (8704)

_cached = None


def _build():
    from contextlib import ExitStack

    import concourse.tile as tile
    from concourse import bacc, mybir

    F32 = mybir.dt.float32
    BF16 = mybir.dt.bfloat16
    AF = mybir.ActivationFunctionType
    OP = mybir.AluOpType

    nc = bacc.Bacc("TRN2", target_bir_lowering=False, debug=False,
                   num_devices=NCORES)

    # E-pack along the free dim (one DMA for all E-partition consts):
    #   gt [E,B] | db [E,2,HC] | ow2 [E,2,L,HC] | ob [E,L] | gtz [E,P]
    EPACK = B + OC + L * OC + L + P      # 64+128+256+2+128 = 578
    xw_d = nc.dram_tensor("xw", [P, XWC], BF16, kind="ExternalInput")
    ep_d = nc.dram_tensor("ep", [E, EPACK], BF16, kind="ExternalInput")
    gc_d = nc.dram_tensor("gc", [P, E], F32, kind="ExternalInput")
    out_d = nc.dram_tensor("out", [P, L], F32, kind="ExternalOutput")

    with tile.TileContext(nc) as tc, ExitStack() as ctx:
        consts = ctx.enter_context(tc.tile_pool(name="consts", bufs=1))
        wpool = ctx.enter_context(tc.tile_pool(name="wpool", bufs=1))
        mixp = ctx.enter_context(tc.tile_pool(name="mixp", bufs=1))
        smallp = ctx.enter_context(tc.tile_pool(name="smallp", bufs=1))
        psy = ctx.enter_context(tc.tile_pool(name="psy", bufs=1, space="PSUM"))
        pss = ctx.enter_context(tc.tile_pool(name="pss", bufs=1, space="PSUM"))

        # Weight stream: ALL weights on the sync ring - the two HW-DGE
        # rings share one descriptor-processing pool (~420 GB/s aggregate;
        # the scalar ring only gets service after sync's queued work), so
        # splitting buys no bandwidth and the scalar ring is further
        # handicapped by the tanh ACT_TABLE_LOAD's DMA leading its queue.
        # 4 chunks (xt+k0k1k2 | k3k4k5 | k6 | k7): few enough boundaries
        # to hold ~390 GB/s, front-loaded first chunk so fewer k-pairs
        # serialize behind later chunk completions, small final chunk so
        # the exposed completion lag (sem16 = bytes + ~1.1us) covers only
        # one k-pair.
        # The scalar ring carries just gc+ep (tiny, consumed late).
        xw_t = wpool.tile([P, XWC], BF16)
        nc.sync.dma_start(out=xw_t[:, 0:XTC + 3072],
                          in_=xw_d.ap()[:, 0:XTC + 3072])
        # ep leads the scalar ring: the tile scheduler parks the ep-gated
        # small matmuls ahead of ready k-tiles in the in-order PE queue,
        # so ep's completion must beat chunk 0's (~11.4us).  gc's consumer
        # (the gb build) is not needed until the mix (~16.5us).
        ep_t = consts.tile([E, EPACK], BF16)
        nc.scalar.dma_start(out=ep_t, in_=ep_d.ap())
        gc_t = consts.tile([P, E], F32)
        nc.scalar.dma_start(out=gc_t, in_=gc_d.ap())
        nc.sync.dma_start(out=xw_t[:, XTC + 3072:XTC + 6144],
                          in_=xw_d.ap()[:, XTC + 3072:XTC + 6144])
        nc.sync.dma_start(out=xw_t[:, XTC + 6144:XTC + 7168],
                          in_=xw_d.ap()[:, XTC + 6144:XTC + 7168])
        nc.sync.dma_start(out=xw_t[:, XTC + 7168:XWC],
                          in_=xw_d.ap()[:, XTC + 7168:XWC])
        xt_t = xw_t[:, 0:XTC].rearrange("p (k b) -> p k b", k=KT)
        w1_t = xw_t[:, XTC:XWC].rearrange("p (k h e c) -> p k h e c",
                                          k=KT, h=2, e=E)
        o = 0
        gt_t = ep_t[:, o:o + B]; o += B
        db_t = ep_t[:, o:o + OC].rearrange("e (h c) -> e h c", h=2); o += OC
        ow_t = ep_t[:, o:o + L * OC].rearrange(
            "e (h l c) -> e h l c", h=2, l=L); o += L * OC
        ob_t = ep_t[:, o:o + L]; o += L
        gtz_t = ep_t[:, o:o + P]                 # gates.T | zeros

        # gate-broadcast table gb[p, (e, hc)] = g[b, e], built early on the
        # DVE (hidden under the w1 stream).
        ones_t = smallp.tile([P, HC], F32)
        nc.vector.memset(ones_t[:], 1.0)
        gb_t = consts.tile([P, E, HC], F32)
        for e in range(E):
            nc.vector.tensor_scalar_mul(gb_t[:, e, :], ones_t[:],
                                        gc_t[:, e:e + 1])

        # ---- stage 1: y[64h+b, (e, hc)] = x . dense_w[e, oc_half, :] ----
        # h=0 / h=1 matmuls write PSUM partition halves 0-63 / 64-127 =
        # disjoint PE col-groups -> concurrent streams.  k-outer so the PE
        # consumes each chunk as it lands.
        psum_y = psy.tile([P, E, HC], F32)

        def big_matmul(k):
            for h in range(2):
                nc.tensor.matmul(
                    psum_y[h * 64:h * 64 + 64, :, :].rearrange(
                        "b e c -> b (e c)"),
                    xt_t[:, k, :],
                    w1_t[:, k, h].rearrange("p e c -> p (e c)"),
                    start=(k == 0),
                    stop=(k == KT - 1),
                    skip_group_check=True,
                )

        for k in range(KT):
            big_matmul(k)

        # ---- small matmuls after the k-loop (the in-order PE queue must
        # never park a not-yet-ready ep-dependent matmul in front of ready
        # k-tiles; after k7 they only need to beat the vector tree) ----
        # sel_db^h [64h+b, hc] ; sel_ow^h [64h+b, (l, hc)] ; sel_ob [p, l]
        psum_db = pss.tile([P, HC], F32)
        psum_ow = pss.tile([P, L, HC], F32)
        for h in range(2):
            sl = slice(h * 64, h * 64 + 64)
            nc.tensor.matmul(psum_db[sl, :], gt_t, db_t[:, h, :],
                             start=True, stop=True, skip_group_check=True)
            nc.tensor.matmul(
                psum_ow[sl, :, :].rearrange("b l c -> b (l c)"),
                gt_t, ow_t[:, h].rearrange("e l c -> e (l c)"),
                start=True, stop=True, skip_group_check=True,
            )
        psum_ob = pss.tile([P, L], F32)
        nc.tensor.matmul(psum_ob[:], gtz_t, ob_t, start=True, stop=True)
        # Copies run on the scalar engine (ACT COPY), which is idle from
        # the early gc/ep dispatches until tanh - keeping them off the
        # vector queue keeps the mix tree's critical path clean.
        sdb_t = smallp.tile([P, HC], BF16)
        nc.scalar.copy(sdb_t[:], psum_db[:])
        # SBUF copy so stage 2 reads SBUF; col HC carries sel_ob so stage
        # 2's accumulate emits the final logits.
        sow_t = smallp.tile([P, L, HC + 1], BF16)
        nc.scalar.copy(sow_t[:, :, 0:HC], psum_ow[:])
        nc.scalar.copy(
            sow_t[:, :, HC:HC + 1].rearrange("p l c -> p (l c)"), psum_ob[:])

        # ---- gate mix: y[p, hc] = sum_e gb[p,e,:] * psum_y[p,e,:] ----
        # contiguous pairwise tree over e; bf16 intermediates double the
        # DVE rate on the adds.
        prod_t = mixp.tile([P, E, HC], BF16)
        nc.vector.tensor_tensor(
            out=prod_t[:], in0=psum_y[:], in1=gb_t[:], op=OP.mult,
        )
        t1 = mixp.tile([P, 4, HC], BF16)
        nc.vector.tensor_add(t1[:], prod_t[:, 0:4, :], prod_t[:, 4:8, :])
        t2 = mixp.tile([P, 2, HC], BF16)
        nc.vector.tensor_add(t2[:], t1[:, 0:2, :], t1[:, 2:4, :])
        t3 = mixp.tile([P, HC], BF16)
        nc.vector.tensor_add(t3[:], t2[:, 0, :], t2[:, 1, :])
        acc = mixp.tile([P, HC], BF16)
        nc.vector.tensor_add(acc[:], t3[:], sdb_t[:])

        # tanh into cols 0:HC of t_ext; col HC holds 1.0 so stage 2's
        # accumulate picks up sel_ob.
        t_ext = smallp.tile([P, HC + 1], BF16)
        nc.gpsimd.memset(t_ext[:, HC:HC + 1], 1.0)
        nc.scalar.activation(t_ext[:, 0:HC], acc[:], AF.Tanh)

        # ---- stage 2: out[64h+b, l] = sum_hc t * sel_ow + sel_ob ----
        # NOTE: InstTensorTensorReduce faults TRN2 and scalar_tensor_tensor
        # only exists on the DVE; accum_out (free-dim sum) there is the
        # reliable path, one op per label column.
        # out_sb is a RAW sbuf tensor (concrete AP), so the post-context
        # output DMA below can reference it without symbolic-AP lowering.
        out_sb = nc.alloc_sbuf_tensor("out_sb", [P, L], F32)
        dumpv = smallp.tile([P, HC + 1], BF16)
        for l in range(L):
            nc.vector.scalar_tensor_tensor(
                out=dumpv[:], in0=sow_t[:, l, :], scalar=1.0, in1=t_ext[:],
                op0=OP.mult, op1=OP.mult, accum_out=out_sb.ap()[:, l:l + 1])

    # Output DMA OUTSIDE the tile context: the tile teardown's sync-drain
    # would otherwise serially wait ~3us for the out DMA's 16 descriptor
    # completions before the (fixed, ~7us) NEFF epilogue.  Emitted here it
    # runs after the tile end barrier (which orders it after out_t's
    # producer on the DVE) and its completion hides under the epilogue's
    # per-engine semaphore-reset cascade; the 1 KiB lands ~1.5us after
    # dispatch, several us before the NEFF's final barrier retires.
    out_sem = nc.alloc_semaphore("out_dma_sem")
    nc.sync.dma_start(
        out=out_d.ap(), in_=out_sb.ap(), single_packet=True,
    ).then_inc(out_sem, 16)

    nc.compile()
    return nc


def _prep_inputs(X, gates, dense_w, dense_b, out_w, out_b):
    """Host-side layout prep (slice/transpose/cast only) -> per-core maps."""
    X = np.asarray(X, dtype=np.float32)
    gates = np.asarray(gates, dtype=np.float32)
    dense_w = np.asarray(dense_w, dtype=np.float32)
    dense_b = np.asarray(dense_b, dtype=np.float32)
    out_w = np.asarray(out_w, dtype=np.float32)
    out_b = np.asarray(out_b, dtype=np.float32)

    xcls = X[:, 0, :]                                     # [B, H]
    # xt[i_lo, k, b] = x[b, k*128 + i_lo]
    xt = (xcls.T.reshape(KT, P, B).transpose(1, 0, 2)
          .reshape(P, XTC).astype(bfloat16))
    gt = gates.T                                          # [E, B]
    gtz = np.concatenate([gt, np.zeros_like(gt)], axis=1)  # [E, 128]
    gc2 = np.ascontiguousarray(np.vstack([gates, gates]))  # [128, E] fp32

    in_maps = []
    for c in range(NCORES):
        sl = slice(c * OC, (c + 1) * OC)
        # w1[i_lo, k, h, e, hc] = dense_w[e, c*OC + h*64 + hc, k*128 + i_lo]
        w1 = (dense_w[:, sl, :]                 # [E, OC, H]
              .reshape(E, 2, HC, KT, P)         # [e, h, hc, k, i_lo]
              .transpose(4, 3, 1, 0, 2)         # [i_lo, k, h, e, hc]
              .reshape(P, W1C))
        xw = np.concatenate([xt, w1.astype(bfloat16)], axis=1)

        # ow2[e, (h, l, hc)] = out_w[e, l, c*OC + h*64 + hc]
        ow2 = (out_w[:, :, sl].reshape(E, L, 2, HC)
               .transpose(0, 2, 1, 3).reshape(E, L * OC))
        ob = out_b if c == 0 else np.zeros_like(out_b)
        ep = np.concatenate(
            [gt, dense_b[:, sl], ow2, ob, gtz], axis=1).astype(bfloat16)
        in_maps.append({
            "xw": np.ascontiguousarray(xw),
            "ep": np.ascontiguousarray(ep),
            "gc": gc2,
        })
    return in_maps


def _run(in_maps, trace=False, tmpdir=None):
    global _cached
    from concourse.bass_utils import run_bass_kernel_spmd

    if _cached is None:
        _cached = _build()
    res = run_bass_kernel_spmd(
        _cached, in_maps, list(range(NCORES)), trace=trace, tmpdir=tmpdir,
    )
    return res


def kernel(X, gates, dense_w, dense_b, out_w, out_b):
    in_maps = _prep_inputs(X, gates, dense_w, dense_b, out_w, out_b)
    res = _run(in_maps)
    acc = np.zeros((B, L), dtype=np.float64)
    for c in range(NCORES):
        part = res.results[c]["out"].astype(np.float64)   # [128, L]
        acc += part.reshape(2, B, L).sum(axis=0)
    return acc.astype(np.float32)


# revision 25
# speedup vs baseline: 1.0029x; 1.0029x over previous
"""Trainium2 Bass kernel for nn_MoEsparseRoutingForClassification.

Reference computation (B=64, S=128, H=1024, E=8, L=2):
    x = X[:, 0, :]                                   # CLS token [B,H]
    y[b,o]   = sum_e g[b,e] * (x[b] . dense_w[e,o,:]) + (g @ dense_b)[b,o]
    t        = tanh(y)
    out[b,l] = sum_e g[b,e] * (t[b] . out_w[e,l,:])  + (g @ out_b)[b,l]

Distribution: the H output dim of the dense layer is sharded 8 ways
(OC=128 per core).  Core c computes y[:, c*OC:(c+1)*OC], applies tanh,
and contracts against out_w[:, :, c_slice] for a partial [B,L] logit;
partials sum on the host.  No cross-core collective.

All streamed tensors are bf16 (host cast; tolerance 2e-2, bf16 lands
~5e-3): HBM traffic halves to ~2.2 MiB/core and every matmul is a
single-pass bf16 op.  The CLS block (xt) and w1 ride ONE packed DRAM
tensor, streamed on the sync ring in 4 chunks the PE chases.

Scheduling notes (from NTFF traces; measured 22.9us vs 31.8us for the
fp32 predecessor, of which ~2.5us head and ~7us NEFF-teardown cascade
are fixed framework costs inside the measured exec window):
  - the two HW-DGE rings (sync/SP + scalar/Activation) share one
    descriptor-processing pool (~420 GB/s aggregate, and the scalar
    ring only gets service after sync's queued work), so weights all
    go on the sync ring; more than ~4 chunk boundaries costs ~25%
    stream rate;
  - every DMA's completion semaphore reaches its 16-increment target
    ~1.1us AFTER its last byte (fixed descriptor-completion pacing),
    so consumers see chunk-done = bytes + ~1.1us; the final k7-only
    chunk keeps that exposed lag to one k-pair;
  - tiny-descriptor transfers (ep/gc) stall a weight queue mid-stream
    and complete very late on the handicapped scalar ring - gc+ep ride
    the scalar ring, whose first ~2us are eaten by the tanh
    ACT_TABLE_LOAD's own DMA anyway;
  - gpsimd TENSOR_SCALAR is ~1.15us/op (5x vector) and gpsimd cannot
    touch PSUM - gb builds on the DVE, where it hides under the stream;
  - the 1 KiB output DMA is emitted AFTER the TileContext closes
    (raw sbuf staging tensor + manual .then_inc sem): the tile
    teardown would otherwise serially wait ~3us for its descriptor
    completions; untracked, they hide under the teardown's per-engine
    semaphore-reset cascade, and the tile end barrier still orders the
    dispatch after the final DVE op.  The NEFF's last instruction
    retires ~6us after the bytes land, so the output is quiescent well
    before the runtime reads it.

On-core layout: OC=128 splits into two 64-wide halves mapped to PSUM
partition halves (rows 64h+b); the two matmul streams per k-tile write
disjoint PE column groups which the PE runs concurrently (426ns per
N=512 bf16 pair).

Tail: bf16 pairwise tree for the gate mix; out_b folded into stage 2
via a constant-1 column appended to tanh(y) and sel_ob as column HC of
the sel_ow SBUF copy (copy also lets stage 2 read SBUF, not PSUM).
"""

import sys

import numpy as np

for _p in ("/opt/trn_rl_repo",):
    if _p not in sys.path:
        sys.path.insert(0, _p)

# If the environment sets BASS_TRACE but lacks antenv.axon_hooks (this agent
# image does), run_bass_kernel_spmd would crash on import; pre-seed a no-op
# module so tracing degrades gracefully instead.
try:  # pragma: no cover
    import antenv.axon_hooks  # noqa: F401
except Exception:  # pragma: no cover
    import types as _types

    _m = _types.ModuleType("antenv.axon_hooks")
    _m._hook = None
    _m.set_axon_ntff_profile_hook = lambda h: setattr(_m, "_hook", h)
    _m.get_axon_ntff_profile_hook = lambda: _m._hook
    sys.modules["antenv.axon_hooks"] = _m

from ml_dtypes import bfloat16  # noqa: E402

B, S, H = 64, 128, 1024
E, L = 8, 2
NCORES = 8
OC = H // NCORES          # dense-output slice per core (128)
HC = OC // 2              # half-slice mapped to a PSUM partition half (64)
KT = H // 128             # contraction tiles
P = 128

XTC = KT * B              # xt cols in the packed stream (512)
W1C = KT * 2 * E * HC     # w1 cols (8192)
XWC = XTC + W1C           # packed stream width (8704)

_cached = None


def _build():
    from contextlib import ExitStack

    import concourse.tile as tile
    from concourse import bacc, mybir

    F32 = mybir.dt.float32
    BF16 = mybir.dt.bfloat16
    AF = mybir.ActivationFunctionType
    OP = mybir.AluOpType

    nc = bacc.Bacc("TRN2", target_bir_lowering=False, debug=False,
                   num_devices=NCORES)

    # E-pack along the free dim (one DMA for all E-partition consts):
    #   gt [E,B] | db [E,2,HC] | ow2 [E,2,L,HC] | ob [E,L] | gtz [E,P]
    EPACK = B + OC + L * OC + L + P      # 64+128+256+2+128 = 578
    xw_d = nc.dram_tensor("xw", [P, XWC], BF16, kind="ExternalInput")
    ep_d = nc.dram_tensor("ep", [E, EPACK], BF16, kind="ExternalInput")
    gc_d = nc.dram_tensor("gc", [P, E], F32, kind="ExternalInput")
    out_d = nc.dram_tensor("out", [P, L], F32, kind="ExternalOutput")

    with tile.TileContext(nc) as tc, ExitStack() as ctx:
        consts = ctx.enter_context(tc.tile_pool(name="consts", bufs=1))
        wpool = ctx.enter_context(tc.tile_pool(name="wpool", bufs=1))
        mixp = ctx.enter_context(tc.tile_pool(name="mixp", bufs=1))
        smallp = ctx.enter_context(tc.tile_pool(name="smallp", bufs=1))
        psy = ctx.enter_context(tc.tile_pool(name="psy", bufs=1, space="PSUM"))
        pss = ctx.enter_context(tc.tile_pool(name="pss", bufs=1, space="PSUM"))

        # Weight stream: ALL weights on the sync ring - the two HW-DGE
        # rings share one descriptor-processing pool (~420 GB/s aggregate;
        # the scalar ring only gets service after sync's queued work), so
        # splitting buys no bandwidth and the scalar ring is further
        # handicapped by the tanh ACT_TABLE_LOAD's DMA leading its queue.
        # 4 chunks (xt+k0k1k2 | k3k4 | k5k6 | k7): few enough boundaries
        # to hold ~390 GB/s, front-loaded first chunk so fewer k-pairs
        # serialize behind later chunk completions, small final chunk so
        # the exposed completion lag (sem16 = bytes + ~1.1us) covers only
        # one k-pair.
        # The scalar ring carries just gc+ep (tiny, consumed late).
        xw_t = wpool.tile([P, XWC], BF16)
        nc.sync.dma_start(out=xw_t[:, 0:XTC + 3072],
                          in_=xw_d.ap()[:, 0:XTC + 3072])
        # ep leads the scalar ring: the tile scheduler parks the ep-gated
        # small matmuls ahead of ready k-tiles in the in-order PE queue,
        # so ep's completion must beat chunk 0's (~11.4us).  gc's consumer
        # (the gb build) is not needed until the mix (~16.5us).
        ep_t = consts.tile([E, EPACK], BF16)
        nc.scalar.dma_start(out=ep_t, in_=ep_d.ap())
        gc_t = consts.tile([P, E], F32)
        nc.scalar.dma_start(out=gc_t, in_=gc_d.ap())
        nc.sync.dma_start(out=xw_t[:, XTC + 3072:XTC + 5120],
                          in_=xw_d.ap()[:, XTC + 3072:XTC + 5120])
        nc.sync.dma_start(out=xw_t[:, XTC + 5120:XTC + 7168],
                          in_=xw_d.ap()[:, XTC + 5120:XTC + 7168])
        nc.sync.dma_start(out=xw_t[:, XTC + 7168:XWC],
                          in_=xw_d.ap()[:, XTC + 7168:XWC])
        xt_t = xw_t[:, 0:XTC].rearrange("p (k b) -> p k b", k=KT)
        w1_t = xw_t[:, XTC:XWC].rearrange("p (k h e c) -> p k h e c",
                                          k=KT, h=2, e=E)
        o = 0
        gt_t = ep_t[:, o:o + B]; o += B
        db_t = ep_t[:, o:o + OC].rearrange("e (h c) -> e h c", h=2); o += OC
        ow_t = ep_t[:, o:o + L * OC].rearrange(
            "e (h l c) -> e h l c", h=2, l=L); o += L * OC
        ob_t = ep_t[:, o:o + L]; o += L
        gtz_t = ep_t[:, o:o + P]                 # gates.T | zeros

        # gate-broadcast table gb[p, (e, hc)] = g[b, e], built early on the
        # DVE (hidden under the w1 stream).
        ones_t = smallp.tile([P, HC], F32)
        nc.vector.memset(ones_t[:], 1.0)
        gb_t = consts.tile([P, E, HC], F32)
        for e in range(E):
            nc.vector.tensor_scalar_mul(gb_t[:, e, :], ones_t[:],
                                        gc_t[:, e:e + 1])

        # ---- stage 1: y[64h+b, (e, hc)] = x . dense_w[e, oc_half, :] ----
        # h=0 / h=1 matmuls write PSUM partition halves 0-63 / 64-127 =
        # disjoint PE col-groups -> concurrent streams.  k-outer so the PE
        # consumes each chunk as it lands.
        psum_y = psy.tile([P, E, HC], F32)

        def big_matmul(k):
            for h in range(2):
                nc.tensor.matmul(
                    psum_y[h * 64:h * 64 + 64, :, :].rearrange(
                        "b e c -> b (e c)"),
                    xt_t[:, k, :],
                    w1_t[:, k, h].rearrange("p e c -> p (e c)"),
                    start=(k == 0),
                    stop=(k == KT - 1),
                    skip_group_check=True,
                )

        for k in range(KT):
            big_matmul(k)

        # ---- small matmuls after the k-loop (the in-order PE queue must
        # never park a not-yet-ready ep-dependent matmul in front of ready
        # k-tiles; after k7 they only need to beat the vector tree) ----
        # sel_db^h [64h+b, hc] ; sel_ow^h [64h+b, (l, hc)] ; sel_ob [p, l]
        psum_db = pss.tile([P, HC], F32)
        psum_ow = pss.tile([P, L, HC], F32)
        for h in range(2):
            sl = slice(h * 64, h * 64 + 64)
            nc.tensor.matmul(psum_db[sl, :], gt_t, db_t[:, h, :],
                             start=True, stop=True, skip_group_check=True)
            nc.tensor.matmul(
                psum_ow[sl, :, :].rearrange("b l c -> b (l c)"),
                gt_t, ow_t[:, h].rearrange("e l c -> e (l c)"),
                start=True, stop=True, skip_group_check=True,
            )
        psum_ob = pss.tile([P, L], F32)
        nc.tensor.matmul(psum_ob[:], gtz_t, ob_t, start=True, stop=True)
        # Copies run on the scalar engine (ACT COPY), which is idle from
        # the early gc/ep dispatches until tanh - keeping them off the
        # vector queue keeps the mix tree's critical path clean.
        sdb_t = smallp.tile([P, HC], BF16)
        nc.scalar.copy(sdb_t[:], psum_db[:])
        # SBUF copy so stage 2 reads SBUF; col HC carries sel_ob so stage
        # 2's accumulate emits the final logits.
        sow_t = smallp.tile([P, L, HC + 1], BF16)
        nc.scalar.copy(sow_t[:, :, 0:HC], psum_ow[:])
        nc.scalar.copy(
            sow_t[:, :, HC:HC + 1].rearrange("p l c -> p (l c)"), psum_ob[:])

        # ---- gate mix: y[p, hc] = sum_e gb[p,e,:] * psum_y[p,e,:] ----
        # contiguous pairwise tree over e; bf16 intermediates double the
        # DVE rate on the adds.
        prod_t = mixp.tile([P, E, HC], BF16)
        nc.vector.tensor_tensor(
            out=prod_t[:], in0=psum_y[:], in1=gb_t[:], op=OP.mult,
        )
        t1 = mixp.tile([P, 4, HC], BF16)
        nc.vector.tensor_add(t1[:], prod_t[:, 0:4, :], prod_t[:, 4:8, :])
        t2 = mixp.tile([P, 2, HC], BF16)
        nc.vector.tensor_add(t2[:], t1[:, 0:2, :], t1[:, 2:4, :])
        t3 = mixp.tile([P, HC], BF16)
        nc.vector.tensor_add(t3[:], t2[:, 0, :], t2[:, 1, :])
        acc = mixp.tile([P, HC], BF16)
        nc.vector.tensor_add(acc[:], t3[:], sdb_t[:])

        # tanh into cols 0:HC of t_ext; col HC holds 1.0 so stage 2's
        # accumulate picks up sel_ob.
        t_ext = smallp.tile([P, HC + 1], BF16)
        nc.gpsimd.memset(t_ext[:, HC:HC + 1], 1.0)
        nc.scalar.activation(t_ext[:, 0:HC], acc[:], AF.Tanh)

        # ---- stage 2: out[64h+b, l] = sum_hc t * sel_ow + sel_ob ----
        # NOTE: InstTensorTensorReduce faults TRN2 and scalar_tensor_tensor
        # only exists on the DVE; accum_out (free-dim sum) there is the
        # reliable path, one op per label column.
        # out_sb is a RAW sbuf tensor (concrete AP), so the post-context
        # output DMA below can reference it without symbolic-AP lowering.
        out_sb = nc.alloc_sbuf_tensor("out_sb", [P, L], F32)
        dumpv = smallp.tile([P, HC + 1], BF16)
        for l in range(L):
            nc.vector.scalar_tensor_tensor(
                out=dumpv[:], in0=sow_t[:, l, :], scalar=1.0, in1=t_ext[:],
                op0=OP.mult, op1=OP.mult, accum_out=out_sb.ap()[:, l:l + 1])

    # Output DMA OUTSIDE the tile context: the tile teardown's sync-drain
    # would otherwise serially wait ~3us for the out DMA's 16 descriptor
    # completions before the (fixed, ~7us) NEFF epilogue.  Emitted here it
    # runs after the tile end barrier (which orders it after out_t's
    # producer on the DVE) and its completion hides under the epilogue's
    # per-engine semaphore-reset cascade; the 1 KiB lands ~1.5us after
    # dispatch, several us before the NEFF's final barrier retires.
    out_sem = nc.alloc_semaphore("out_dma_sem")
    nc.sync.dma_start(
        out=out_d.ap(), in_=out_sb.ap(), single_packet=True,
    ).then_inc(out_sem, 16)

    nc.compile()
    return nc


def _prep_inputs(X, gates, dense_w, dense_b, out_w, out_b):
    """Host-side layout prep (slice/transpose/cast only) -> per-core maps."""
    X = np.asarray(X, dtype=np.float32)
    gates = np.asarray(gates, dtype=np.float32)
    dense_w = np.asarray(dense_w, dtype=np.float32)
    dense_b = np.asarray(dense_b, dtype=np.float32)
    out_w = np.asarray(out_w, dtype=np.float32)
    out_b = np.asarray(out_b, dtype=np.float32)

    xcls = X[:, 0, :]                                     # [B, H]
    # xt[i_lo, k, b] = x[b, k*128 + i_lo]
    xt = (xcls.T.reshape(KT, P, B).transpose(1, 0, 2)
          .reshape(P, XTC).astype(bfloat16))
    gt = gates.T                                          # [E, B]
    gtz = np.concatenate([gt, np.zeros_like(gt)], axis=1)  # [E, 128]
    gc2 = np.ascontiguousarray(np.vstack([gates, gates]))  # [128, E] fp32

    in_maps = []
    for c in range(NCORES):
        sl = slice(c * OC, (c + 1) * OC)
        # w1[i_lo, k, h, e, hc] = dense_w[e, c*OC + h*64 + hc, k*128 + i_lo]
        w1 = (dense_w[:, sl, :]                 # [E, OC, H]
              .reshape(E, 2, HC, KT, P)         # [e, h, hc, k, i_lo]
              .transpose(4, 3, 1, 0, 2)         # [i_lo, k, h, e, hc]
              .reshape(P, W1C))
        xw = np.concatenate([xt, w1.astype(bfloat16)], axis=1)

        # ow2[e, (h, l, hc)] = out_w[e, l, c*OC + h*64 + hc]
        ow2 = (out_w[:, :, sl].reshape(E, L, 2, HC)
               .transpose(0, 2, 1, 3).reshape(E, L * OC))
        ob = out_b if c == 0 else np.zeros_like(out_b)
        ep = np.concatenate(
            [gt, dense_b[:, sl], ow2, ob, gtz], axis=1).astype(bfloat16)
        in_maps.append({
            "xw": np.ascontiguousarray(xw),
            "ep": np.ascontiguousarray(ep),
            "gc": gc2,
        })
    return in_maps


def _run(in_maps, trace=False, tmpdir=None):
    global _cached
    from concourse.bass_utils import run_bass_kernel_spmd

    if _cached is None:
        _cached = _build()
    res = run_bass_kernel_spmd(
        _cached, in_maps, list(range(NCORES)), trace=trace, tmpdir=tmpdir,
    )
    return res


def kernel(X, gates, dense_w, dense_b, out_w, out_b):
    in_maps = _prep_inputs(X, gates, dense_w, dense_b, out_w, out_b)
    res = _run(in_maps)
    acc = np.zeros((B, L), dtype=np.float64)
    for c in range(NCORES):
        part = res.results[c]["out"].astype(np.float64)   # [128, L]
        acc += part.reshape(2, B, L).sum(axis=0)
    return acc.astype(np.float32)


# revision 28
# speedup vs baseline: 1.0062x; 1.0033x over previous
"""Trainium2 Bass kernel for nn_MoEsparseRoutingForClassification.

Reference computation (B=64, S=128, H=1024, E=8, L=2):
    x = X[:, 0, :]                                   # CLS token [B,H]
    y[b,o]   = sum_e g[b,e] * (x[b] . dense_w[e,o,:]) + (g @ dense_b)[b,o]
    t        = tanh(y)
    out[b,l] = sum_e g[b,e] * (t[b] . out_w[e,l,:])  + (g @ out_b)[b,l]

Distribution: the H output dim of the dense layer is sharded 8 ways
(OC=128 per core).  Core c computes y[:, c*OC:(c+1)*OC], applies tanh,
and contracts against out_w[:, :, c_slice] for a partial [B,L] logit;
partials sum on the host.  No cross-core collective.

All streamed tensors are bf16 (host cast; tolerance 2e-2, bf16 lands
~5e-3): HBM traffic halves to ~2.2 MiB/core and every matmul is a
single-pass bf16 op.  The CLS block (xt) and w1 ride ONE packed DRAM
tensor, streamed on the sync ring in 4 chunks the PE chases.

Scheduling notes (from NTFF traces; measured 22.9us vs 31.8us for the
fp32 predecessor, of which ~2.5us head and ~7us NEFF-teardown cascade
are fixed framework costs inside the measured exec window):
  - the two HW-DGE rings (sync/SP + scalar/Activation) share one
    descriptor-processing pool (~420 GB/s aggregate, and the scalar
    ring only gets service after sync's queued work), so weights all
    go on the sync ring; more than ~4 chunk boundaries costs ~25%
    stream rate;
  - every DMA's completion semaphore reaches its 16-increment target
    ~1.1us AFTER its last byte (fixed descriptor-completion pacing),
    so consumers see chunk-done = bytes + ~1.1us; the final k7-only
    chunk keeps that exposed lag to one k-pair;
  - tiny-descriptor transfers (ep/gc) stall a weight queue mid-stream
    and complete very late on the handicapped scalar ring - gc+ep ride
    the scalar ring, whose first ~2us are eaten by the tanh
    ACT_TABLE_LOAD's own DMA anyway;
  - gpsimd TENSOR_SCALAR is ~1.15us/op (5x vector) and gpsimd cannot
    touch PSUM - gb builds on the DVE, where it hides under the stream;
  - the 1 KiB output DMA is emitted AFTER the TileContext closes
    (raw sbuf staging tensor + manual .then_inc sem): the tile
    teardown would otherwise serially wait ~3us for its descriptor
    completions; untracked, they hide under the teardown's per-engine
    semaphore-reset cascade, and the tile end barrier still orders the
    dispatch after the final DVE op.  The NEFF's last instruction
    retires ~6us after the bytes land, so the output is quiescent well
    before the runtime reads it.

On-core layout: OC=128 splits into two 64-wide halves mapped to PSUM
partition halves (rows 64h+b); the two matmul streams per k-tile write
disjoint PE column groups which the PE runs concurrently (426ns per
N=512 bf16 pair).

Tail: bf16 pairwise tree for the gate mix; out_b folded into stage 2
via a constant-1 column appended to tanh(y) and sel_ob as column HC of
the sel_ow SBUF copy (copy also lets stage 2 read SBUF, not PSUM).
"""

import sys

import numpy as np

for _p in ("/opt/trn_rl_repo",):
    if _p not in sys.path:
        sys.path.insert(0, _p)

# If the environment sets BASS_TRACE but lacks antenv.axon_hooks (this agent
# image does), run_bass_kernel_spmd would crash on import; pre-seed a no-op
# module so tracing degrades gracefully instead.
try:  # pragma: no cover
    import antenv.axon_hooks  # noqa: F401
except Exception:  # pragma: no cover
    import types as _types

    _m = _types.ModuleType("antenv.axon_hooks")
    _m._hook = None
    _m.set_axon_ntff_profile_hook = lambda h: setattr(_m, "_hook", h)
    _m.get_axon_ntff_profile_hook = lambda: _m._hook
    sys.modules["antenv.axon_hooks"] = _m

from ml_dtypes import bfloat16  # noqa: E402

B, S, H = 64, 128, 1024
E, L = 8, 2
NCORES = 8
OC = H // NCORES          # dense-output slice per core (128)
HC = OC // 2              # half-slice mapped to a PSUM partition half (64)
KT = H // 128             # contraction tiles
P = 128

XTC = KT * B              # xt cols in the packed stream (512)
W1C = KT * 2 * E * HC     # w1 cols (8192)
XWC = XTC + W1C           # packed stream width (8704)

_cached = None


def _build():
    from contextlib import ExitStack

    import concourse.tile as tile
    from concourse import bacc, mybir

    F32 = mybir.dt.float32
    BF16 = mybir.dt.bfloat16
    AF = mybir.ActivationFunctionType
    OP = mybir.AluOpType

    nc = bacc.Bacc("TRN2", target_bir_lowering=False, debug=False,
                   num_devices=NCORES)

    # E-pack along the free dim (one DMA for all E-partition consts):
    #   gt [E,B] | db [E,2,HC] | ow2 [E,2,L,HC] | ob [E,L] | gtz [E,P]
    EPACK = B + OC + L * OC + L + P      # 64+128+256+2+128 = 578
    # + a 1024-col dense-bias row block in PARTITION 0 ONLY (matmul
    # operands must sit at base partition 0/32/64): cols h*512+(e,hc) =
    # dense_b[e, h*64+hc], for the K=1 bias matmul
    EPACK2 = EPACK + 2 * E * HC          # 578+1024
    xw_d = nc.dram_tensor("xw", [P, XWC], BF16, kind="ExternalInput")
    ep_d = nc.dram_tensor("ep", [E, EPACK2], BF16, kind="ExternalInput")
    gc_d = nc.dram_tensor("gc", [P, E], F32, kind="ExternalInput")
    out_d = nc.dram_tensor("out", [P, L], F32, kind="ExternalOutput")

    with tile.TileContext(nc) as tc, ExitStack() as ctx:
        consts = ctx.enter_context(tc.tile_pool(name="consts", bufs=1))
        wpool = ctx.enter_context(tc.tile_pool(name="wpool", bufs=1))
        mixp = ctx.enter_context(tc.tile_pool(name="mixp", bufs=1))
        smallp = ctx.enter_context(tc.tile_pool(name="smallp", bufs=1))
        psy = ctx.enter_context(tc.tile_pool(name="psy", bufs=1, space="PSUM"))
        pss = ctx.enter_context(tc.tile_pool(name="pss", bufs=1, space="PSUM"))

        # Weight stream: ALL weights on the sync ring - the two HW-DGE
        # rings share one descriptor-processing pool (~420 GB/s aggregate;
        # the scalar ring only gets service after sync's queued work), so
        # splitting buys no bandwidth and the scalar ring is further
        # handicapped by the tanh ACT_TABLE_LOAD's DMA leading its queue.
        # 4 chunks (xt+k0k1k2 | k3k4 | k5k6 | k7): few enough boundaries
        # to hold ~390 GB/s, front-loaded first chunk so fewer k-pairs
        # serialize behind later chunk completions, small final chunk so
        # the exposed completion lag (sem16 = bytes + ~1.1us) covers only
        # one k-pair.
        # The scalar ring carries just gc+ep (tiny, consumed late).
        xw_t = wpool.tile([P, XWC], BF16)
        nc.sync.dma_start(out=xw_t[:, 0:XTC + 3072],
                          in_=xw_d.ap()[:, 0:XTC + 3072])
        # ep leads the scalar ring: the tile scheduler parks the ep-gated
        # small matmuls ahead of ready k-tiles in the in-order PE queue,
        # so ep's completion must beat chunk 0's (~11.4us).  gc's consumer
        # (the gb build) is not needed until the mix (~16.5us).
        ep_t = consts.tile([E, EPACK2], BF16)
        nc.scalar.dma_start(out=ep_t, in_=ep_d.ap())
        gc_t = consts.tile([P, E], F32)
        nc.scalar.dma_start(out=gc_t, in_=gc_d.ap())
        nc.sync.dma_start(out=xw_t[:, XTC + 3072:XTC + 5120],
                          in_=xw_d.ap()[:, XTC + 3072:XTC + 5120])
        nc.sync.dma_start(out=xw_t[:, XTC + 5120:XTC + 7168],
                          in_=xw_d.ap()[:, XTC + 5120:XTC + 7168])
        nc.sync.dma_start(out=xw_t[:, XTC + 7168:XWC],
                          in_=xw_d.ap()[:, XTC + 7168:XWC])
        xt_t = xw_t[:, 0:XTC].rearrange("p (k b) -> p k b", k=KT)
        w1_t = xw_t[:, XTC:XWC].rearrange("p (k h e c) -> p k h e c",
                                          k=KT, h=2, e=E)
        o = 0
        gt_t = ep_t[:, o:o + B]; o += B
        db_t = ep_t[:, o:o + OC].rearrange("e (h c) -> e h c", h=2); o += OC
        ow_t = ep_t[:, o:o + L * OC].rearrange(
            "e (h l c) -> e h l c", h=2, l=L); o += L * OC
        ob_t = ep_t[:, o:o + L]; o += L
        gtz_t = ep_t[:, o:o + P]; o += P         # gates.T | zeros
        dbr_t = ep_t[:, o:o + 2 * E * HC]        # partition-0 bias rows
        ones1_t = smallp.tile([1, B], BF16)
        nc.vector.memset(ones1_t[:], 1.0)

        # gate-broadcast table gb[p, (e, hc)] = g[b, e], built early on the
        # DVE (hidden under the w1 stream).
        ones_t = smallp.tile([P, HC], F32)
        nc.vector.memset(ones_t[:], 1.0)
        gb_t = consts.tile([P, E, HC], F32)
        for e in range(E):
            nc.vector.tensor_scalar_mul(gb_t[:, e, :], ones_t[:],
                                        gc_t[:, e:e + 1])

        # ---- stage 1: y[64h+b, (e, hc)] = x . dense_w[e, oc_half, :] ----
        # h=0 / h=1 matmuls write PSUM partition halves 0-63 / 64-127 =
        # disjoint PE col-groups -> concurrent streams.  k-outer so the PE
        # consumes each chunk as it lands.
        psum_y = psy.tile([P, E, HC], F32)

        def big_matmul(k):
            for h in range(2):
                nc.tensor.matmul(
                    psum_y[h * 64:h * 64 + 64, :, :].rearrange(
                        "b e c -> b (e c)"),
                    xt_t[:, k, :],
                    w1_t[:, k, h].rearrange("p e c -> p (e c)"),
                    start=(k == 0),
                    stop=(k == KT - 1),
                    skip_group_check=True,
                )

        big_matmul(0)
        big_matmul(1)
        # dense_b rides the PSUM group as a K=1 matmul (ones row x bias
        # row), slotted into the k1->k2 PE bubble - the gate mix then
        # emits y + sel_db directly and the tail loses its +sdb add.
        for h in range(2):
            nc.tensor.matmul(
                psum_y[h * 64:h * 64 + 64, :, :].rearrange("b e c -> b (e c)"),
                ones1_t[:],
                dbr_t[0:1, h * E * HC:(h + 1) * E * HC],
                start=False, stop=False, skip_group_check=True,
            )
        for k in range(2, KT):
            big_matmul(k)

        # ---- small matmuls after the k-loop (the in-order PE queue must
        # never park a not-yet-ready ep-dependent matmul in front of ready
        # k-tiles; after k7 they only need to beat the vector tree) ----
        # sel_db^h [64h+b, hc] ; sel_ow^h [64h+b, (l, hc)] ; sel_ob [p, l]
        psum_ow = pss.tile([P, L, HC], F32)
        for h in range(2):
            sl = slice(h * 64, h * 64 + 64)
            nc.tensor.matmul(
                psum_ow[sl, :, :].rearrange("b l c -> b (l c)"),
                gt_t, ow_t[:, h].rearrange("e l c -> e (l c)"),
                start=True, stop=True, skip_group_check=True,
            )
        psum_ob = pss.tile([P, L], F32)
        nc.tensor.matmul(psum_ob[:], gtz_t, ob_t, start=True, stop=True)
        # Copies run on the scalar engine (ACT COPY), which is idle from
        # the early gc/ep dispatches until tanh - keeping them off the
        # vector queue keeps the mix tree's critical path clean.
        # SBUF copy so stage 2 reads SBUF; col HC carries sel_ob so stage
        # 2's accumulate emits the final logits.
        sow_t = smallp.tile([P, L, HC + 1], BF16)
        nc.scalar.copy(sow_t[:, :, 0:HC], psum_ow[:])
        nc.scalar.copy(
            sow_t[:, :, HC:HC + 1].rearrange("p l c -> p (l c)"), psum_ob[:])

        # ---- gate mix: y[p, hc] = sum_e gb[p,e,:] * psum_y[p,e,:] ----
        # contiguous pairwise tree over e; bf16 intermediates double the
        # DVE rate on the adds.
        prod_t = mixp.tile([P, E, HC], BF16)
        nc.vector.tensor_tensor(
            out=prod_t[:], in0=psum_y[:], in1=gb_t[:], op=OP.mult,
        )
        t1 = mixp.tile([P, 4, HC], BF16)
        nc.vector.tensor_add(t1[:], prod_t[:, 0:4, :], prod_t[:, 4:8, :])
        t2 = mixp.tile([P, 2, HC], BF16)
        nc.vector.tensor_add(t2[:], t1[:, 0:2, :], t1[:, 2:4, :])
        acc = mixp.tile([P, HC], BF16)
        nc.vector.tensor_add(acc[:], t2[:, 0, :], t2[:, 1, :])

        # tanh into cols 0:HC of t_ext; col HC holds 1.0 so stage 2's
        # accumulate picks up sel_ob.
        t_ext = smallp.tile([P, HC + 1], BF16)
        nc.gpsimd.memset(t_ext[:, HC:HC + 1], 1.0)
        nc.scalar.activation(t_ext[:, 0:HC], acc[:], AF.Tanh)

        # ---- stage 2: out[64h+b, l] = sum_hc t * sel_ow + sel_ob ----
        # NOTE: InstTensorTensorReduce faults TRN2 and scalar_tensor_tensor
        # only exists on the DVE; accum_out (free-dim sum) there is the
        # reliable path, one op per label column.
        # out_sb is a RAW sbuf tensor (concrete AP), so the post-context
        # output DMA below can reference it without symbolic-AP lowering.
        out_sb = nc.alloc_sbuf_tensor("out_sb", [P, L], F32)
        dumpv = smallp.tile([P, HC + 1], BF16)
        for l in range(L):
            nc.vector.scalar_tensor_tensor(
                out=dumpv[:], in0=sow_t[:, l, :], scalar=1.0, in1=t_ext[:],
                op0=OP.mult, op1=OP.mult, accum_out=out_sb.ap()[:, l:l + 1])

    # Output DMA OUTSIDE the tile context: the tile teardown's sync-drain
    # would otherwise serially wait ~3us for the out DMA's 16 descriptor
    # completions before the (fixed, ~7us) NEFF epilogue.  Emitted here it
    # runs after the tile end barrier (which orders it after out_t's
    # producer on the DVE) and its completion hides under the epilogue's
    # per-engine semaphore-reset cascade; the 1 KiB lands ~1.5us after
    # dispatch, several us before the NEFF's final barrier retires.
    out_sem = nc.alloc_semaphore("out_dma_sem")
    nc.sync.dma_start(
        out=out_d.ap(), in_=out_sb.ap(), single_packet=True,
    ).then_inc(out_sem, 16)

    nc.compile()
    return nc


def _prep_inputs(X, gates, dense_w, dense_b, out_w, out_b):
    """Host-side layout prep (slice/transpose/cast only) -> per-core maps."""
    X = np.asarray(X, dtype=np.float32)
    gates = np.asarray(gates, dtype=np.float32)
    dense_w = np.asarray(dense_w, dtype=np.float32)
    dense_b = np.asarray(dense_b, dtype=np.float32)
    out_w = np.asarray(out_w, dtype=np.float32)
    out_b = np.asarray(out_b, dtype=np.float32)

    xcls = X[:, 0, :]                                     # [B, H]
    # xt[i_lo, k, b] = x[b, k*128 + i_lo]
    xt = (xcls.T.reshape(KT, P, B).transpose(1, 0, 2)
          .reshape(P, XTC).astype(bfloat16))
    gt = gates.T                                          # [E, B]
    gtz = np.concatenate([gt, np.zeros_like(gt)], axis=1)  # [E, 128]
    gc2 = np.ascontiguousarray(np.vstack([gates, gates]))  # [128, E] fp32

    in_maps = []
    for c in range(NCORES):
        sl = slice(c * OC, (c + 1) * OC)
        # w1[i_lo, k, h, e, hc] = dense_w[e, c*OC + h*64 + hc, k*128 + i_lo]
        w1 = (dense_w[:, sl, :]                 # [E, OC, H]
              .reshape(E, 2, HC, KT, P)         # [e, h, hc, k, i_lo]
              .transpose(4, 3, 1, 0, 2)         # [i_lo, k, h, e, hc]
              .reshape(P, W1C))
        xw = np.concatenate([xt, w1.astype(bfloat16)], axis=1)

        # ow2[e, (h, l, hc)] = out_w[e, l, c*OC + h*64 + hc]
        ow2 = (out_w[:, :, sl].reshape(E, L, 2, HC)
               .transpose(0, 2, 1, 3).reshape(E, L * OC))
        ob = out_b if c == 0 else np.zeros_like(out_b)
        # dbrow[h, (e, hc)] = dense_b[e, c*OC + h*64 + hc], in E-partition
        # rows 0/1 (rows 2..E-1 zero) for the K=1 bias matmul.
        dbrow = np.zeros((E, 2 * E * HC), dtype=np.float32)
        dbrow[0] = (dense_b[:, sl].reshape(E, 2, HC)
                    .transpose(1, 0, 2).reshape(2 * E * HC))
        ep = np.concatenate(
            [gt, dense_b[:, sl], ow2, ob, gtz, dbrow],
            axis=1).astype(bfloat16)
        in_maps.append({
            "xw": np.ascontiguousarray(xw),
            "ep": np.ascontiguousarray(ep),
            "gc": gc2,
        })
    return in_maps


def _run(in_maps, trace=False, tmpdir=None):
    global _cached
    from concourse.bass_utils import run_bass_kernel_spmd

    if _cached is None:
        _cached = _build()
    res = run_bass_kernel_spmd(
        _cached, in_maps, list(range(NCORES)), trace=trace, tmpdir=tmpdir,
    )
    return res


def kernel(X, gates, dense_w, dense_b, out_w, out_b):
    in_maps = _prep_inputs(X, gates, dense_w, dense_b, out_w, out_b)
    res = _run(in_maps)
    acc = np.zeros((B, L), dtype=np.float64)
    for c in range(NCORES):
        part = res.results[c]["out"].astype(np.float64)   # [128, L]
        acc += part.reshape(2, B, L).sum(axis=0)
    return acc.astype(np.float32)
